# revision 6
# baseline (speedup 1.0000x reference)
"""Trainium2 Bass kernel for nn_CrossScalePeriodicFeatureAggregator.

Reference computation (per expert e with patch size p_e, L_e = 336 / p_e):
    h = einsum('nld,pd->nlp', xs_e, W_e) + b_e      # [128, L_e, p_e*512]
    h -> reshape [128, 336, 512]                     # seq-stitch
    proj = h @ Wp.T + bp                             # shared projection
    out[batch_index] += gate * proj                  # gated scatter-combine

Kernel strategy (8 cores, SPMD):
  * Algebraic fusion: the two chained matmuls collapse into one. For output
    position s = l*p_e + q:  out[n, s, :] = x[n, l, :] @ WF_e[q]  where
    WF_e[q] = W_e[q*512:(q+1)*512, :].T @ Wp.T   (precomputed on host).
    This halves device FLOPs (90 GF instead of 180 GF).
  * Gates are folded into x rows on host (mathematically identical).
  * Token sharding: core c takes rows [16c, 16c+16) of every expert
    -> perfectly balanced (each core: 16*336 tokens per expert through a
    512->512 matmul per q), single uniform SPMD program.
  * Matmuls run in float32r (TF32-like, full PE rate at N=512, measured
    rel-err ~1.4e-4 vs the fp32 reference for K=512 contractions).
  * Device writes per-expert projections out_e[q, t, :] (q-major, large
    contiguous DMAs); host de-interleaves and does the gated combine.
"""
import numpy as np

PATCH = [4, 8, 12, 24]
SEQ = 336
D = 512
NE = 4
BATCH = 256
ROWS_PER_EXPERT = 128
N_CORES = 8
ROWS_PER_CORE = ROWS_PER_EXPERT // N_CORES          # 16
L = [SEQ // p for p in PATCH]                       # [84, 42, 28, 14]
T = [ROWS_PER_CORE * l for l in L]                  # tokens/core: [1344, 672, 448, 224]
NT = [(t + 127) // 128 for t in T]                  # token tiles: [11, 6, 4, 2]
KC = 4                                              # contraction chunks of 128

_CACHED = {}


def _build_nc(reps=1):
    """reps>1 replays the whole compute body (for differential HW timing)."""
    import concourse.mybir as mybir
    from concourse import bacc
    from concourse.tile import TileContext

    f32r = mybir.dt.float32r
    f32 = mybir.dt.float32

    nc = bacc.Bacc("TRN2", target_bir_lowering=False, debug=False,
                   num_devices=N_CORES)
    xt = [nc.dram_tensor(f"xt{e}", [128, KC * T[e]], f32r, kind="ExternalInput")
          for e in range(NE)]
    wf = [nc.dram_tensor(f"wf{e}", [PATCH[e], 128, KC * D], f32r,
                         kind="ExternalInput") for e in range(NE)]
    # token dim padded to NT*128; tail rows of the last (partial) tile carry
    # garbage and are sliced off on the host
    out = [nc.dram_tensor(f"out{e}", [PATCH[e], NT[e] * 128, D], f32,
                          kind="ExternalOutput") for e in range(NE)]

    MAXNT = max(NT)
    with TileContext(nc) as tc:
        with (
            tc.tile_pool(name="xpool", bufs=1) as xpool,
            tc.tile_pool(name="wpool", bufs=8) as wpool,
            tc.tile_pool(name="spool", bufs=2) as spool,
            tc.tile_pool(name="ppool", bufs=8, space="PSUM") as ppool,
        ):
            xtiles = []
            for e in range(NE):
                t = xpool.tile([128, KC * T[e]], f32r, tag=f"xt{e}")
                nc.sync.dma_start(t[:], xt[e].ap())
                xtiles.append(t)

            copy_flip = 0
            for _rep in range(reps):
                for e in range(NE):
                    p = PATCH[e]
                    for q in range(p):
                        wt = wpool.tile([128, KC * D], f32r, tag="wt")
                        nc.sync.dma_start(wt[:], wf[e].ap()[q])
                        st = spool.tile([128, MAXNT * D], f32, tag="st")
                        for mt in range(NT[e]):
                            m = min(128, T[e] - 128 * mt)
                            ps = ppool.tile([128, D], f32)
                            for k in range(KC):
                                nc.tensor.matmul(
                                    ps[:m, :],
                                    xtiles[e][:, k * T[e] + 128 * mt:
                                              k * T[e] + 128 * mt + m],
                                    wt[:, k * D:(k + 1) * D],
                                    start=(k == 0), stop=(k == KC - 1),
                                )
                            dst = st[:m, mt * D:(mt + 1) * D]
                            if copy_flip % 2 == 0:
                                nc.scalar.copy(dst, ps[:m, :])
                            else:
                                nc.vector.tensor_copy(dst, ps[:m, :])
                            copy_flip += 1
                        # one DMA for this (e, q): rows t = mt*128 + j
                        src = st[:, :NT[e] * D].rearrange(
                            "j (mt d) -> j mt d", d=D)
                        dst_ap = out[e].ap()[q].rearrange(
                            "(mt j) d -> j mt d", j=128)
                        nc.sync.dma_start(dst_ap, src)
    nc.compile()
    return nc


def _get_nc():
    if "nc" not in _CACHED:
        _CACHED["nc"] = _build_nc()
    return _CACHED["nc"]


def _prep(xs, Ws, gates, Wp, batch_index, expert_index):
    """Host-side shard prep. Returns (in_maps, row_of_expert, g_row)."""
    row_of_expert = [np.nonzero(expert_index == e)[0] for e in range(NE)]
    g_row = gates[batch_index, expert_index].astype(np.float32)   # [NNZ]

    # Fused weights WF_e[q] = W_e[q*512:(q+1)*512, :].T @ Wp.T  -> [c, d_out];
    # device layout wf_e[q, p, k*512+d] with c = 128k + p.
    wf_in = []
    for e in range(NE):
        p = PATCH[e]
        w = Ws[e].reshape(p, D, D)                     # [q, d_mid, c]
        WF = np.einsum("qdc,od->qco", w, Wp, optimize=True)   # [q, c, d_out]
        wf_in.append(np.ascontiguousarray(
            WF.reshape(p, KC, 128, D).transpose(0, 2, 1, 3)   # [q, p128, k, d]
              .reshape(p, 128, KC * D)))

    in_maps = []
    for c in range(N_CORES):
        m = {}
        for e in range(NE):
            rows = slice(c * ROWS_PER_CORE, (c + 1) * ROWS_PER_CORE)
            gr = g_row[row_of_expert[e][rows]]
            x = xs[e][rows] * gr[:, None, None]        # [16, L, 512]
            x = x.reshape(T[e], D)                     # tokens
            # xt[p, k*T + t] = x[t, 128k + p]
            m[f"xt{e}"] = np.ascontiguousarray(
                x.reshape(T[e], KC, 128).transpose(2, 1, 0)
                 .reshape(128, KC * T[e]))
            m[f"wf{e}"] = wf_in[e]
        in_maps.append(m)
    return in_maps, row_of_expert, g_row


def _combine(results, row_of_expert, batch_index):
    """De-interleave q-major device outputs and gated-combine per batch."""
    combined = np.zeros((BATCH, SEQ, D), np.float32)
    for e in range(NE):
        p = PATCH[e]
        full = np.empty((ROWS_PER_EXPERT, SEQ, D), np.float32)
        for c in range(N_CORES):
            dev = results[c][f"out{e}"][:, :T[e], :]   # [p, T_e, D]
            # out_seq[r, l*p + q, :] = dev[q, r*L + l, :]
            blk = dev.reshape(p, ROWS_PER_CORE, L[e], D).transpose(1, 2, 0, 3)
            full[c * ROWS_PER_CORE:(c + 1) * ROWS_PER_CORE] = \
                blk.reshape(ROWS_PER_CORE, SEQ, D)
        bids = batch_index[row_of_expert[e]]
        if len(np.unique(bids)) == len(bids):
            combined[bids] += full
        else:
            np.add.at(combined, bids, full)
    return combined


def kernel(xs0, xs1, xs2, xs3, gates, W0, b0, W1, b1, W2, b2, W3, b3, Wp, bp,
           batch_index, expert_index, _collect_results=None):
    from concourse.bass_utils import run_bass_kernel_spmd

    xs = [np.asarray(x, np.float32) for x in (xs0, xs1, xs2, xs3)]
    Ws = [np.asarray(w, np.float32) for w in (W0, W1, W2, W3)]
    bs = [np.asarray(b, np.float32) for b in (b0, b1, b2, b3)]
    gates = np.asarray(gates, np.float32)
    Wp = np.asarray(Wp, np.float32)
    bp = np.asarray(bp, np.float32)
    batch_index = np.asarray(batch_index)
    expert_index = np.asarray(expert_index)

    in_maps, row_of_expert, g_row = _prep(xs, Ws, gates, Wp,
                                          batch_index, expert_index)
    nc = _get_nc()
    res = run_bass_kernel_spmd(nc, in_maps, list(range(N_CORES)))
    if _collect_results is not None:
        _collect_results.append(res)

    combined = _combine(res.results, row_of_expert, batch_index)

    # Bias terms (zero in this problem's inputs; handled for correctness).
    if any(np.any(b) for b in bs) or np.any(bp):
        for e in range(NE):
            p = PATCH[e]
            bF = bs[e].reshape(p, D) @ Wp.T + bp       # [q, d_out]
            bias_seq = np.tile(bF, (L[e], 1)).reshape(SEQ, D)
            bids = batch_index[row_of_expert[e]]
            gr = g_row[row_of_expert[e]]
            contrib = gr[:, None, None] * bias_seq[None]
            if len(np.unique(bids)) == len(bids):
                combined[bids] += contrib
            else:
                np.add.at(combined, bids, contrib)

    return combined


# revision 12
# speedup vs baseline: 5371.5004x; 5371.5004x over previous
"""Trainium2 Bass kernel for nn_CrossScalePeriodicFeatureAggregator.

Reference computation (per expert e with patch size p_e, L_e = 336 / p_e):
    h = einsum('nld,pd->nlp', xs_e, W_e) + b_e      # [128, L_e, p_e*512]
    h -> reshape [128, 336, 512]                     # seq-stitch
    proj = h @ Wp.T + bp                             # shared projection
    out[batch_index] += gate * proj                  # gated scatter-combine

Kernel strategy (8 cores, SPMD):
  * Algebraic fusion: the two chained matmuls collapse into one. For output
    position s = l*p_e + q:  out[n, s, :] = x[n, l, :] @ WF_e[q]  where
    WF_e[q] = W_e[q*512:(q+1)*512, :].T @ Wp.T   (precomputed on host).
    This halves device FLOPs (90 GF instead of 180 GF).
  * Gates are folded into x rows on host (mathematically identical).
  * Token sharding: core c takes rows [16c, 16c+16) of every expert
    -> perfectly balanced (each core: 16*336 tokens per expert through a
    512->512 matmul per q), single uniform SPMD program.
  * Matmuls run in float32r (TF32-like, full PE rate at N=512, measured
    rel-err ~1.4e-4 vs the fp32 reference for K=512 contractions).
  * Device writes per-expert projections out_e[q, t, :] (q-major, large
    contiguous DMAs); host de-interleaves and does the gated combine.
"""
import numpy as np

PATCH = [4, 8, 12, 24]
SEQ = 336
D = 512
NE = 4
BATCH = 256
ROWS_PER_EXPERT = 128
N_CORES = 8
ROWS_PER_CORE = ROWS_PER_EXPERT // N_CORES          # 16
L = [SEQ // p for p in PATCH]                       # [84, 42, 28, 14]
T = [ROWS_PER_CORE * l for l in L]                  # tokens/core: [1344, 672, 448, 224]
NT = [(t + 127) // 128 for t in T]                  # token tiles: [11, 6, 4, 2]
KC = 4                                              # contraction chunks of 128

_CACHED = {}


def _build_nc(loop_n=0, internal_wf=False):
    """loop_n>0 wraps the compute body in a hardware For_i loop (differential
    HW timing); internal_wf sources weights from internal DRAM (skips the
    host upload, contents irrelevant for timing)."""
    import concourse.mybir as mybir
    from concourse import bacc
    from concourse.tile import TileContext

    f32r = mybir.dt.float32r
    f32 = mybir.dt.float32

    nc = bacc.Bacc("TRN2", target_bir_lowering=False, debug=False,
                   num_devices=N_CORES)
    xt = [nc.dram_tensor(f"xt{e}", [128, KC * T[e]], f32r, kind="ExternalInput")
          for e in range(NE)]
    if internal_wf:
        wf = [nc.dram_tensor(f"iwf{e}", [PATCH[e], 128, KC * D], f32r)
              for e in range(NE)]
    else:
        wf = [nc.dram_tensor(f"wf{e}", [PATCH[e], 128, KC * D], f32r,
                             kind="ExternalInput") for e in range(NE)]
    # partition-major layout [q, j, mt*D]: token t = mt*128 + j, so every
    # SBUF partition writes one contiguous run. Tail tokens >= T[e] are
    # garbage from the partial tile and are sliced off on the host.
    out = [nc.dram_tensor(f"out{e}", [PATCH[e], 128, NT[e] * D], f32,
                          kind="ExternalOutput") for e in range(NE)]

    MAXNT = max(NT)
    with TileContext(nc) as tc:
        with (
            tc.tile_pool(name="xpool", bufs=1) as xpool,
            tc.tile_pool(name="wpool", bufs=8) as wpool,
            tc.tile_pool(name="spool", bufs=2) as spool,
            tc.tile_pool(name="ppool", bufs=8, space="PSUM") as ppool,
        ):
            xtiles = []
            for e in range(NE):
                t = xpool.tile([128, KC * T[e]], f32r, tag=f"xt{e}")
                nc.sync.dma_start(t[:], xt[e].ap())
                xtiles.append(t)

            # weights stream on the sync HWDGE ring; output stores alternate
            # between the scalar HWDGE ring and the gpsimd SWDGE ring so they
            # never block weight prefetch (FIFO per ring).
            out_engs = [nc.scalar, nc.gpsimd]
            state = {"flip": 0, "oi": 0}

            def body():
                for e in range(NE):
                    for q in range(PATCH[e]):
                        wt = wpool.tile([128, KC * D], f32r, tag="wt")
                        nc.sync.dma_start(wt[:], wf[e].ap()[q])
                        st = spool.tile([128, MAXNT * D], f32, tag="st")
                        for mt in range(NT[e]):
                            m = min(128, T[e] - 128 * mt)
                            ps = ppool.tile([128, D], f32)
                            for k in range(KC):
                                nc.tensor.matmul(
                                    ps[:m, :],
                                    xtiles[e][:, k * T[e] + 128 * mt:
                                              k * T[e] + 128 * mt + m],
                                    wt[:, k * D:(k + 1) * D],
                                    start=(k == 0), stop=(k == KC - 1),
                                )
                            dst = st[:m, mt * D:(mt + 1) * D]
                            if state["flip"] % 2 == 0:
                                nc.scalar.copy(dst, ps[:m, :])
                            else:
                                nc.vector.tensor_copy(dst, ps[:m, :])
                            state["flip"] += 1
                        # one fully-contiguous DMA for this (e, q)
                        out_engs[state["oi"] % 2].dma_start(out[e].ap()[q],
                                                            st[:, :NT[e] * D])
                        state["oi"] += 1

            if loop_n > 0:
                with tc.For_i(0, loop_n, 1):
                    body()
            else:
                body()
    nc.compile()
    return nc


def _get_nc():
    if "nc" not in _CACHED:
        _CACHED["nc"] = _build_nc()
    return _CACHED["nc"]


def _prep(xs, Ws, gates, Wp, batch_index, expert_index):
    """Host-side shard prep. Returns (in_maps, row_of_expert, g_row)."""
    row_of_expert = [np.nonzero(expert_index == e)[0] for e in range(NE)]
    g_row = gates[batch_index, expert_index].astype(np.float32)   # [NNZ]

    # Fused weights WF_e[q] = W_e[q*512:(q+1)*512, :].T @ Wp.T  -> [c, d_out];
    # device layout wf_e[q, p, k*512+d] with c = 128k + p.
    wf_in = []
    for e in range(NE):
        p = PATCH[e]
        w = Ws[e].reshape(p, D, D)                     # [q, d_mid, c]
        WF = np.einsum("qdc,od->qco", w, Wp, optimize=True)   # [q, c, d_out]
        wf_in.append(np.ascontiguousarray(
            WF.reshape(p, KC, 128, D).transpose(0, 2, 1, 3)   # [q, p128, k, d]
              .reshape(p, 128, KC * D)))

    in_maps = []
    for c in range(N_CORES):
        m = {}
        for e in range(NE):
            rows = slice(c * ROWS_PER_CORE, (c + 1) * ROWS_PER_CORE)
            gr = g_row[row_of_expert[e][rows]]
            x = xs[e][rows] * gr[:, None, None]        # [16, L, 512]
            x = x.reshape(T[e], D)                     # tokens
            # xt[p, k*T + t] = x[t, 128k + p]
            m[f"xt{e}"] = np.ascontiguousarray(
                x.reshape(T[e], KC, 128).transpose(2, 1, 0)
                 .reshape(128, KC * T[e]))
            m[f"wf{e}"] = wf_in[e]
        in_maps.append(m)
    return in_maps, row_of_expert, g_row


def _combine(results, row_of_expert, batch_index):
    """De-interleave q-major device outputs and gated-combine per batch."""
    combined = np.zeros((BATCH, SEQ, D), np.float32)
    for e in range(NE):
        p = PATCH[e]
        full = np.empty((ROWS_PER_EXPERT, SEQ, D), np.float32)
        for c in range(N_CORES):
            # device layout [q, j, mt, d]; token t = mt*128 + j
            raw = results[c][f"out{e}"].reshape(p, 128, NT[e], D)
            dev = raw.transpose(0, 2, 1, 3).reshape(p, NT[e] * 128, D)[:, :T[e], :]
            # out_seq[r, l*p + q, :] = dev[q, r*L + l, :]
            blk = dev.reshape(p, ROWS_PER_CORE, L[e], D).transpose(1, 2, 0, 3)
            full[c * ROWS_PER_CORE:(c + 1) * ROWS_PER_CORE] = \
                blk.reshape(ROWS_PER_CORE, SEQ, D)
        bids = batch_index[row_of_expert[e]]
        if len(np.unique(bids)) == len(bids):
            combined[bids] += full
        else:
            np.add.at(combined, bids, full)
    return combined


def kernel(xs0, xs1, xs2, xs3, gates, W0, b0, W1, b1, W2, b2, W3, b3, Wp, bp,
           batch_index, expert_index, _collect_results=None):
    from concourse.bass_utils import run_bass_kernel_spmd

    xs = [np.asarray(x, np.float32) for x in (xs0, xs1, xs2, xs3)]
    Ws = [np.asarray(w, np.float32) for w in (W0, W1, W2, W3)]
    bs = [np.asarray(b, np.float32) for b in (b0, b1, b2, b3)]
    gates = np.asarray(gates, np.float32)
    Wp = np.asarray(Wp, np.float32)
    bp = np.asarray(bp, np.float32)
    batch_index = np.asarray(batch_index)
    expert_index = np.asarray(expert_index)

    in_maps, row_of_expert, g_row = _prep(xs, Ws, gates, Wp,
                                          batch_index, expert_index)
    nc = _get_nc()
    res = run_bass_kernel_spmd(nc, in_maps, list(range(N_CORES)))
    if _collect_results is not None:
        _collect_results.append(res)

    combined = _combine(res.results, row_of_expert, batch_index)

    # Bias terms (zero in this problem's inputs; handled for correctness).
    if any(np.any(b) for b in bs) or np.any(bp):
        for e in range(NE):
            p = PATCH[e]
            bF = bs[e].reshape(p, D) @ Wp.T + bp       # [q, d_out]
            bias_seq = np.tile(bF, (L[e], 1)).reshape(SEQ, D)
            bids = batch_index[row_of_expert[e]]
            gr = g_row[row_of_expert[e]]
            contrib = gr[:, None, None] * bias_seq[None]
            if len(np.unique(bids)) == len(bids):
                combined[bids] += contrib
            else:
                np.add.at(combined, bids, contrib)

    return combined
